# revision 1
# baseline (speedup 1.0000x reference)
"""DeepseekV3 MoE (B=2, S=2048, H=1024, E=16 top-2, I=512, shared IS=1024)
on 8 Trainium2 NeuronCores.

Distribution (expert-parallel, full-I/O contract):
  - Host computes the gate (sigmoid top-2) and dispatches tokens by expert id
    (the "all-to-all" of the sharding hint, done host-side since kernel()
    receives full inputs).
  - Core c runs the SwiGLU MLPs of experts 2c and 2c+1 over their gathered
    tokens (capacity-padded to C columns).
  - The shared expert is split 2-way over its intermediate dim IS=1024:
    cores (2p, 2p+1) each run one I=512 half over tokens [1024p, 1024p+1024);
    the host sums the two partial outputs.  This loads half the shared
    weights per core at the cost of 2x token traffic -- a net byte saving.
  - Host applies the gate combine weights and sums routed + shared.

Device layout: activations stay feature-major (X^T: partition=feature,
free=token) so every matmul uses the weight tile as the stationary operand
and no on-device transposes are needed.  Matmuls run as float32r
(full-rate fp32 mode, 1 cycle/row at moving dim >= 256; measured end-to-end
relative error vs the f32 reference: ~2.5e-4).
"""

import time

import numpy as np

import concourse.bass as bass
import concourse.mybir as mybir
import concourse.tile as tile
from concourse.bass_utils import run_bass_kernel_spmd


# Model dims (hardcoded per the problem spec)
B, S, H = 2, 2048, 1024
E, K = 16, 2
I = 512
IS = 1024
T = B * S
N_CORES = 8
E_LOC = E // N_CORES          # routed experts per core
TSH = T // (N_CORES // 2)     # shared-expert tokens per core pair (1024)
KH = H // 128                 # contraction chunks over H
KI = I // 128                 # contraction chunks over I

F32 = mybir.dt.float32
F32R = mybir.dt.float32r


def _split_sync_waits(nc, maxw=1):
    """This walrus build's setupSyncWait rejects instructions carrying more
    than ~1 semaphore wait.  Hoist excess waits onto same-engine NoOps
    placed immediately before the instruction (same block order => same
    engine program order => identical stall semantics)."""
    uid = 0
    for f in nc.m.functions:
        for bb in f.blocks:
            out = []
            for inst in bb.instructions:
                si = inst.sync_info
                if si is not None and len(si.on_wait) > maxw:
                    waits = list(si.on_wait)
                    for w in waits[:-maxw]:
                        uid += 1
                        out.append(mybir.InstNoOp(
                            name=f"{inst.name}-sw{uid}",
                            opcode="NoOp",
                            engine=inst.engine,
                            ins=[], outs=[],
                            sync_info=mybir.SyncInfo(on_wait=[w], on_update=[]),
                            bass_nofuse=True,
                        ))
                    si.on_wait[:] = waits[-maxw:]
                out.append(inst)
            bb.instructions[:] = out


def _chunks(tok):
    """Split a token count into moving-dim chunks that keep float32r at
    full rate (>=256) and within the fp32 moving-operand max (512)."""
    if tok <= 512:
        return [(0, tok)]
    out = []
    pos = 0
    rem = tok
    while rem > 0:
        w = 512 if rem >= 768 else (rem if rem <= 512 else rem // 2)
        out.append((pos, w))
        pos += w
        rem -= w
    return out


def build_device_program(C, split_waits=True, repeat=1, cfg=None):
    """One SPMD program, identical on every core."""
    nc = bass.Bass()

    xg = nc.declare_dram_parameter("xg", [E_LOC, H, C], F32R, isOutput=False)
    xs = nc.declare_dram_parameter("xs", [H, TSH], F32R, isOutput=False)
    weg = nc.declare_dram_parameter("weg", [E_LOC, H, I], F32R, isOutput=False)
    weu = nc.declare_dram_parameter("weu", [E_LOC, H, I], F32R, isOutput=False)
    wed = nc.declare_dram_parameter("wed", [E_LOC, I, H], F32R, isOutput=False)
    wsg = nc.declare_dram_parameter("wsg", [H, I], F32R, isOutput=False)
    wsu = nc.declare_dram_parameter("wsu", [H, I], F32R, isOutput=False)
    wsd = nc.declare_dram_parameter("wsd", [I, H], F32R, isOutput=False)
    yg = nc.declare_dram_parameter("yg", [E_LOC, H, C], F32, isOutput=True)
    ys = nc.declare_dram_parameter("ys", [H, TSH], F32, isOutput=True)

    if cfg is None:
        cfg = {}
    bufs = dict(xp=16, wgp=16, wup=16, wdp=8, pp=12, gp=6, yp=6,
                psg=2, psu=2, psy=3, store="scalar")
    bufs.update(cfg)

    # Fit the SBUF budget (~206 KB/partition usable here) when C grows
    # beyond 640: xg/p tile slots scale with C, so shrink pool depths in
    # a priority order until the estimate fits.
    slot = max(C * 4, 4096)  # xg tile [128, C] vs xs tile [128, 1024]

    def est():
        return ((bufs["xp"] + bufs["pp"]) * slot
                + (bufs["wgp"] + bufs["wup"]) * 2048 + bufs["wdp"] * 4096
                + bufs["gp"] * 2 * 2048 + bufs["yp"] * 2048)

    shrink = [("xp", 12), ("pp", 8), ("xp", 10), ("pp", 6),
              ("wgp", 12), ("wup", 12), ("wdp", 6), ("gp", 4), ("yp", 4)]
    i = 0
    while est() > 206 * 1024 and i < len(shrink):
        k, v = shrink[i]
        bufs[k] = min(bufs[k], v)
        i += 1

    with tile.TileContext(nc) as tc:
        with (
            tc.tile_pool(name="xp", bufs=bufs["xp"]) as xp,
            tc.tile_pool(name="wgp", bufs=bufs["wgp"]) as wgp,
            tc.tile_pool(name="wup", bufs=bufs["wup"]) as wup,
            tc.tile_pool(name="wdp", bufs=bufs["wdp"]) as wdp,
            tc.tile_pool(name="pp", bufs=bufs["pp"]) as pp,
            tc.tile_pool(name="gp", bufs=bufs["gp"]) as gp,
            tc.tile_pool(name="yp", bufs=bufs["yp"]) as yp,
            tc.tile_pool(name="psg", bufs=bufs["psg"], space="PSUM") as psg,
            tc.tile_pool(name="psu", bufs=bufs["psu"], space="PSUM") as psu,
            tc.tile_pool(name="psy", bufs=bufs["psy"], space="PSUM") as psy,
        ):

            def load_chunks(pool, dram2d, n_k, width):
                tiles = []
                for k in range(n_k):
                    t = pool.tile([128, width], F32R)
                    nc.sync.dma_start(t[:], dram2d[k * 128:(k + 1) * 128, :])
                    tiles.append(t)
                return tiles

            def swiglu_job(segments, wg_dram, wu_dram, wd_dram):
                """One I=512 SwiGLU MLP over a list of token segments
                (x_dram, out_dram, tok); weights are loaded once."""
                wg_t = load_chunks(wgp, wg_dram, KH, I)
                wu_t = load_chunks(wup, wu_dram, KH, I)
                wd_t = load_chunks(wdp, wd_dram, KI, H)
                for (x_dram, out_dram, tok) in segments:
                    chunks = _chunks(tok)
                    x_t = load_chunks(xp, x_dram, KH, tok)
                    p_tiles = []
                    for i_t in range(KI):
                        p = pp.tile([128, tok], F32R)
                        isl = slice(i_t * 128, (i_t + 1) * 128)
                        for (n0, nw) in chunks:
                            nsl = slice(n0, n0 + nw)
                            g_ps = psg.tile([128, nw], F32)
                            for k in range(KH):
                                nc.tensor.matmul(
                                    g_ps[:], wg_t[k][:, isl], x_t[k][:, nsl],
                                    start=(k == 0), stop=(k == KH - 1),
                                )
                            u_ps = psu.tile([128, nw], F32)
                            for k in range(KH):
                                nc.tensor.matmul(
                                    u_ps[:], wu_t[k][:, isl], x_t[k][:, nsl],
                                    start=(k == 0), stop=(k == KH - 1),
                                )
                            # silu(g)*u: sigmoid + two muls (CoreSim lacks Silu)
                            sg = gp.tile([128, nw], F32)
                            nc.scalar.activation(
                                sg[:], g_ps[:],
                                mybir.ActivationFunctionType.Sigmoid,
                            )
                            gs = gp.tile([128, nw], F32)
                            nc.vector.tensor_mul(gs[:], g_ps[:], sg[:])
                            nc.vector.tensor_mul(p[:, nsl], gs[:], u_ps[:])
                        p_tiles.append(p)

                    for h in range(KH):
                        hsl = slice(h * 128, (h + 1) * 128)
                        for (n0, nw) in chunks:
                            nsl = slice(n0, n0 + nw)
                            y_ps = psy.tile([128, nw], F32)
                            for ki in range(KI):
                                nc.tensor.matmul(
                                    y_ps[:], wd_t[ki][:, hsl],
                                    p_tiles[ki][:, nsl],
                                    start=(ki == 0), stop=(ki == KI - 1),
                                )
                            y_sb = yp.tile([128, nw], F32)
                            nc.vector.tensor_copy(y_sb[:], y_ps[:])
                            store_eng = getattr(nc, bufs.get("store", "sync"))
                            store_eng.dma_start(out_dram[hsl, nsl], y_sb[:])

            half = TSH // 2
            shared_segs = ([(xs, ys, TSH)] if bufs.get("shared_seg") == 1 else
                           [(xs[:, :half], ys[:, :half], half),
                            (xs[:, half:], ys[:, half:], half)])
            for _rep in range(repeat):
                jobs = [([(xg[j], yg[j], C)], weg[j], weu[j], wed[j])
                        for j in range(E_LOC)]
                jobs.append((shared_segs, wsg, wsu, wsd))
                if bufs.get("shared_first"):
                    jobs = jobs[-1:] + jobs[:-1]
                for segs, a, b, d in jobs:
                    swiglu_job(segs, a, b, d)

    if split_waits:
        _split_sync_waits(nc)
    return nc


def _route(x2, gate_weight):
    """Replicate the reference gate: sigmoid scores, top-2 (ties -> lower
    index), normalized weights.  float64 internally for stable ranking."""
    logits = x2.astype(np.float64) @ gate_weight.astype(np.float64).T
    scores = 1.0 / (1.0 + np.exp(-logits))
    topk_idx = np.argsort(-scores, axis=1, kind="stable")[:, :K]
    topk_w = np.take_along_axis(scores, topk_idx, axis=1)
    topk_w = topk_w / (topk_w.sum(-1, keepdims=True) + 1e-20)
    return topk_idx.astype(np.int64), topk_w.astype(np.float32)


def kernel(hidden_states, gate_weight, We_gate, We_up, We_down,
           Ws_gate, Ws_up, Ws_down):
    hidden_states = np.asarray(hidden_states, dtype=np.float32)
    gate_weight = np.asarray(gate_weight, dtype=np.float32)
    We_gate = np.asarray(We_gate, dtype=np.float32)
    We_up = np.asarray(We_up, dtype=np.float32)
    We_down = np.asarray(We_down, dtype=np.float32)
    Ws_gate = np.asarray(Ws_gate, dtype=np.float32)
    Ws_up = np.asarray(Ws_up, dtype=np.float32)
    Ws_down = np.asarray(Ws_down, dtype=np.float32)

    x2 = hidden_states.reshape(T, H)
    topk_idx, topk_w = _route(x2, gate_weight)

    # Dispatch: group the T*K (token, slot) assignments by expert.
    assign = topk_idx.ravel()                       # [T*K]
    order = np.argsort(assign, kind="stable")       # slots grouped by expert
    counts = np.bincount(assign, minlength=E)
    starts = np.concatenate([[0], np.cumsum(counts)[:-1]])
    pos = np.empty(T * K, np.int64)                 # slot within its expert
    pos[order] = np.arange(T * K) - starts[assign[order]]

    C = max(640, int(-(-counts.max() // 128)) * 128)  # capacity, mult of 128

    nc = build_device_program(C)

    xT = x2.T  # [H, T] view; column slices below copy what they need
    in_maps = []
    for c in range(N_CORES):
        pair, half = divmod(c, 2)
        xg_np = np.zeros((E_LOC, H, C), np.float32)
        for j in range(E_LOC):
            e = E_LOC * c + j
            toks = order[starts[e]:starts[e] + counts[e]] // K
            xg_np[j, :, :counts[e]] = x2[toks].T
        in_maps.append({
            "xg": xg_np,
            "xs": np.ascontiguousarray(xT[:, TSH * pair:TSH * (pair + 1)]),
            "weg": np.ascontiguousarray(We_gate[E_LOC * c:E_LOC * (c + 1)]),
            "weu": np.ascontiguousarray(We_up[E_LOC * c:E_LOC * (c + 1)]),
            "wed": np.ascontiguousarray(We_down[E_LOC * c:E_LOC * (c + 1)]),
            "wsg": np.ascontiguousarray(Ws_gate[:, I * half:I * (half + 1)]),
            "wsu": np.ascontiguousarray(Ws_up[:, I * half:I * (half + 1)]),
            "wsd": np.ascontiguousarray(Ws_down[I * half:I * (half + 1), :]),
        })

    # The execution stack occasionally reports a transient device error
    # (e.g. NRT_EXEC_UNIT_UNRECOVERABLE through axon) that clears on the
    # next attempt; retry a couple of times before giving up.
    last_exc = None
    for attempt in range(3):
        try:
            res = run_bass_kernel_spmd(
                nc, in_maps, core_ids=list(range(N_CORES)))
            break
        except Exception as exc:  # noqa: BLE001 - deliberate broad retry
            last_exc = exc
            if attempt == 2:
                raise
            time.sleep(5.0)
            nc = build_device_program(C)

    # Gather: per-expert outputs are [H, C] feature-major -> [E, C, H]
    flat_y = np.empty((E, C, H), np.float32)
    ys_all = np.empty((T, H), np.float32)
    for c in range(N_CORES):
        r = res.results[c]
        for j in range(E_LOC):
            flat_y[E_LOC * c + j] = r["yg"][j].T
    for pair in range(N_CORES // 2):
        ysum = res.results[2 * pair]["ys"] + res.results[2 * pair + 1]["ys"]
        ys_all[TSH * pair:TSH * (pair + 1)] = ysum.T

    yr = (topk_w[:, 0:1] * flat_y[topk_idx[:, 0], pos[0::2]]
          + topk_w[:, 1:2] * flat_y[topk_idx[:, 1], pos[1::2]])

    return (yr + ys_all).reshape(B, S, H).astype(np.float32)

